# revision 1
# baseline (speedup 1.0000x reference)
"""CrossTransformer kernel for Trainium2, data-parallel over batch across 8 cores.

Math per batch b (B=32, N=25, C=512, H=W=14, DK=DV=128):
  qq = Wqk @ Q    [128, 196]      qv = Wv @ Q     [128, 196]
  K  = Wqk @ S    [128, 4900]     V  = Wv @ S     [128, 4900]
  simT[nij, hw] = K^T @ qq        (computed directly in transposed layout)
  E = exp(simT)                   (no max subtraction; |sim| <~ 60 is safe in fp32)
  ctx_raw[hw, v+1] = sum_nij E[nij, hw]^T @ [V^T | 1]   (ones column -> softmax denom)
  ctx = ctx_raw[:, :128] / ctx_raw[:, 128:129]
  partial += sum((qv^T - ctx)^2)
Output per core: scalar partial sum over its 4 batches; host sums and divides by H*W.
"""

import os
import sys

sys.path.insert(0, "/opt/trn_rl_repo")

import numpy as np

import concourse.bass as bass
import concourse.bacc as bacc
import concourse.mybir as mybir
import concourse.tile as tile
from concourse.bass_utils import run_bass_kernel_spmd
from concourse.masks import make_identity

F32 = mybir.dt.float32
F32R = mybir.dt.float32r
BF16 = mybir.dt.bfloat16

B_PER_CORE = 4
N_SUP = 25
C = 512
HW = 196
NIJ = N_SUP * HW  # 4900
DK = 128
NCH = (NIJ + 127) // 128  # 39 chunks of <=128 along nij
FT = 490                   # nij tile width for projections (fits one PSUM bank)
NT = NIJ // FT             # 10
CCH = C // 128             # 4 c-chunks


def _spans(start, end):
    """Split nij range [start,end) into DMA pieces aligned to n boundaries.
    Returns (n0, nn, ij0, L, dstoff) tuples; full-n middle merged into one."""
    res = []
    cur = start
    if cur % HW != 0:
        n = cur // HW
        ij0 = cur % HW
        L = min(HW - ij0, end - cur)
        res.append((n, 1, ij0, L, cur - start))
        cur += L
    nfull = (end - cur) // HW
    if nfull > 0:
        res.append((cur // HW, nfull, 0, HW, cur - start))
        cur += nfull * HW
    if cur < end:
        res.append((cur // HW, 1, 0, end - cur, cur - start))
    return res


def build_bass():
    nc = bacc.Bacc(
        "TRN2", target_bir_lowering=False, debug=False, enable_asserts=False
    )
    q_d = nc.dram_tensor("q", [B_PER_CORE, C, HW], F32, kind="ExternalInput").ap()
    s_d = nc.dram_tensor(
        "s", [B_PER_CORE, N_SUP, C, HW], F32, kind="ExternalInput"
    ).ap()
    wqk_d = nc.dram_tensor("wqk", [DK, C], F32, kind="ExternalInput").ap()
    wv_d = nc.dram_tensor("wv", [DK, C], F32, kind="ExternalInput").ap()
    out_d = nc.dram_tensor("out", [1, 1], F32, kind="ExternalOutput").ap()

    with tile.TileContext(nc) as tc:
        with (
            tc.tile_pool(name="const", bufs=1) as const,
            tc.tile_pool(name="spool", bufs=16) as spool,
            tc.tile_pool(name="kvbf", bufs=2) as kvbf,
            tc.tile_pool(name="vt1p", bufs=2 * NCH + 4) as vt1p,
            tc.tile_pool(name="etp", bufs=NCH + 5) as etp,
            tc.tile_pool(name="small", bufs=4) as small,
            tc.tile_pool(name="ps_proj", bufs=3, space="PSUM") as ps_proj,
            tc.tile_pool(name="ps_sim", bufs=2, space="PSUM") as ps_sim,
            tc.tile_pool(name="ps_vt", bufs=2, space="PSUM") as ps_vt,
            tc.tile_pool(name="ps_ctx", bufs=1, space="PSUM") as ps_ctx,
        ):
            # ---- constants / weights ----
            id_f32 = const.tile([128, 128], F32, tag="id_f32")
            make_identity(nc, id_f32)
            id_bf = const.tile([128, 128], BF16, tag="id_bf")
            make_identity(nc, id_bf)

            wqk_sb = const.tile([128, C], F32, tag="wqk_sb")
            nc.sync.dma_start(out=wqk_sb, in_=wqk_d)
            wv_sb = const.tile([128, C], F32, tag="wv_sb")
            nc.sync.dma_start(out=wv_sb, in_=wv_d)

            wqkT = []
            wvT = []
            for cc in range(CCH):
                for (src, dstl, nm) in ((wqk_sb, wqkT, "qk"), (wv_sb, wvT, "v")):
                    pt = ps_vt.tile([128, 128], F32, tag="ps_vt")
                    nc.tensor.transpose(pt, src[:, cc * 128 : (cc + 1) * 128], id_f32)
                    wt = const.tile([128, 128], F32R, tag=f"w{nm}T{cc}")
                    nc.vector.tensor_copy(wt, pt)
                    dstl.append(wt)

            # ---- query load + projections (all 4 batches at once) ----
            qsb = []
            for cc in range(CCH):
                qt = const.tile([128, B_PER_CORE * HW], F32R, tag=f"qsb{cc}")
                src = q_d[:, cc * 128 : (cc + 1) * 128, :].rearrange(
                    "b c ij -> c b ij"
                ).bitcast(F32R)
                nc.sync.dma_start(
                    out=qt.rearrange("p (b ij) -> p b ij", b=B_PER_CORE), in_=src
                )
                qsb.append(qt)

            qq_bf = const.tile([128, B_PER_CORE * HW], BF16, tag="qq_bf")
            qv_sb = const.tile([128, B_PER_CORE * HW], F32, tag="qv_sb")
            for wT, dst in ((wqkT, qq_bf), (wvT, qv_sb)):
                for half in range(2):
                    hw0 = half * 392
                    pq = ps_proj.tile([128, FT], F32, tag="ps_proj")
                    for cc in range(CCH):
                        nc.tensor.matmul(
                            pq[:, :392],
                            lhsT=wT[cc],
                            rhs=qsb[cc][:, hw0 : hw0 + 392],
                            start=(cc == 0),
                            stop=(cc == CCH - 1),
                        )
                    nc.vector.tensor_copy(dst[:, hw0 : hw0 + 392], pq[:, :392])

            # qv^T per (b, hw-chunk): [hw<=128, 128] fp32 — matches ctx layout
            qvT = {}
            for b in range(B_PER_CORE):
                for h in range(2):
                    hww = 128 if h == 0 else HW - 128
                    pt = ps_vt.tile([128, 128], F32, tag="ps_vt")
                    nc.tensor.transpose(
                        pt[:hww, :],
                        qv_sb[:, b * HW + h * 128 : b * HW + h * 128 + hww],
                        id_f32,
                    )
                    qt = const.tile([128, 128], F32, tag=f"qvT{b}_{h}")
                    nc.vector.tensor_copy(qt[:hww, :], pt[:hww, :])
                    qvT[(b, h)] = qt

            partials = const.tile([128, 2 * B_PER_CORE], F32, tag="partials")
            nc.vector.memset(partials, 0.0)

            # ---- per-batch main pipeline ----
            import os as _os
            KPHASES = int(_os.environ.get("KPHASES", "4"))
            for b in range(B_PER_CORE):
                k_bf = kvbf.tile([128, NIJ], BF16, tag="k_bf")
                v_bf = kvbf.tile([128, NIJ], BF16, tag="v_bf")

                # projections: stream S in FT-wide nij tiles
                for t in range(NT):
                    st = []
                    for cc in range(CCH):
                        s_t = spool.tile([128, FT], F32R, tag="s_t")
                        for (n0, nn, ij0, L, off) in _spans(t * FT, (t + 1) * FT):
                            src = s_d[
                                b, n0 : n0 + nn, cc * 128 : (cc + 1) * 128,
                                ij0 : ij0 + L,
                            ].rearrange("n c ij -> c n ij").bitcast(F32R)
                            nc.sync.dma_start(
                                out=s_t[:, off : off + nn * L].rearrange(
                                    "p (n ij) -> p n ij", n=nn
                                ),
                                in_=src,
                            )
                        st.append(s_t)
                    pk = ps_proj.tile([128, FT], F32, tag="ps_proj")
                    for cc in range(CCH):
                        nc.tensor.matmul(
                            pk,
                            lhsT=wqkT[cc],
                            rhs=st[cc],
                            start=(cc == 0),
                            stop=(cc == CCH - 1),
                        )
                    nc.vector.tensor_copy(k_bf[:, t * FT : (t + 1) * FT], pk)
                    pv = ps_proj.tile([128, FT], F32, tag="ps_proj")
                    for cc in range(CCH):
                        nc.tensor.matmul(
                            pv,
                            lhsT=wvT[cc],
                            rhs=st[cc],
                            start=(cc == 0),
                            stop=(cc == CCH - 1),
                        )
                    nc.scalar.copy(v_bf[:, t * FT : (t + 1) * FT], pv)

                # V^T chunks (+ ones column) via PE transpose
                vt1 = []
                if KPHASES < 2:
                    continue
                for j in range(NCH):
                    cw = min(128, NIJ - j * 128)
                    vt = vt1p.tile([128, 132], BF16, tag="vt1")
                    if cw < 128:
                        nc.vector.memset(vt, 0.0)
                    pt = ps_vt.tile([128, 128], BF16, tag="ps_vt")
                    nc.tensor.transpose(
                        pt[:cw, :], v_bf[:, j * 128 : j * 128 + cw], id_bf
                    )
                    nc.vector.tensor_copy(vt[:cw, 0:128], pt[:cw, :])
                    nc.vector.memset(vt[:, 128:132], 1.0)
                    vt1.append(vt)

                # simT = K^T @ qq (bf16), exp -> E^T chunks
                et = []
                if KPHASES < 3:
                    continue
                for j in range(NCH):
                    cw = min(128, NIJ - j * 128)
                    ps = ps_sim.tile([128, HW], F32, tag="ps_sim")
                    nc.tensor.matmul(
                        ps[:cw, :],
                        lhsT=k_bf[:, j * 128 : j * 128 + cw],
                        rhs=qq_bf[:, b * HW : (b + 1) * HW],
                        start=True,
                        stop=True,
                    )
                    e = etp.tile([128, HW], BF16, tag="et")
                    if cw < 128:
                        nc.vector.memset(e, 0.0)
                    nc.scalar.activation(
                        out=e[:cw, :],
                        in_=ps[:cw, :],
                        func=mybir.ActivationFunctionType.Exp,
                    )
                    et.append(e)

                # PV: ctx_raw[hw, 129] accumulated over 39 nij chunks
                if KPHASES < 4:
                    continue
                for h in range(2):
                    hww = 128 if h == 0 else HW - 128
                    pc = ps_ctx.tile([128, 132], F32, tag="ps_ctx")
                    for j in range(NCH):
                        nc.tensor.matmul(
                            pc[:hww, 0:132],
                            lhsT=et[j][:, h * 128 : h * 128 + hww],
                            rhs=vt1[j][:, 0:132],
                            start=(j == 0),
                            stop=(j == NCH - 1),
                        )
                    r = small.tile([128, 1], F32, tag="recip")
                    nc.vector.reciprocal(r[:hww], pc[:hww, 128:129])
                    ctx = small.tile([128, 128], F32, tag="ctx")
                    nc.vector.tensor_scalar_mul(
                        ctx[:hww, :], pc[:hww, 0:128], r[:hww]
                    )
                    d = small.tile([128, 128], F32, tag="diff")
                    nc.vector.tensor_sub(
                        d[:hww, :], qvT[(b, h)][:hww, :], ctx[:hww, :]
                    )
                    d2 = small.tile([128, 128], F32, tag="d2")
                    nc.vector.tensor_mul(d2[:hww, :], d[:hww, :], d[:hww, :])
                    nc.vector.reduce_sum(
                        partials[:hww, 2 * b + h : 2 * b + h + 1],
                        d2[:hww, :],
                        axis=mybir.AxisListType.X,
                    )

            # ---- final reduction to scalar ----
            tot = small.tile([128, 1], F32, tag="tot")
            nc.vector.reduce_sum(tot, partials, axis=mybir.AxisListType.X)
            ones = small.tile([128, 1], F32, tag="ones")
            nc.vector.memset(ones, 1.0)
            pf = ps_vt.tile([128, 128], F32, tag="ps_vt")
            nc.tensor.matmul(pf[0:1, 0:1], lhsT=tot, rhs=ones, start=True, stop=True)
            ob = small.tile([1, 1], F32, tag="ob")
            nc.vector.tensor_copy(ob, pf[0:1, 0:1])
            nc.sync.dma_start(out=out_d, in_=ob)

    nc.compile()
    return nc


_NC = None


def kernel(query_repr, supports_repr, W_qk, W_v):
    global _NC
    q = np.ascontiguousarray(np.asarray(query_repr, dtype=np.float32)).reshape(
        32, C, HW
    )
    s = np.ascontiguousarray(np.asarray(supports_repr, dtype=np.float32)).reshape(
        32, N_SUP, C, HW
    )
    wqk = np.ascontiguousarray(np.asarray(W_qk, dtype=np.float32))
    wv = np.ascontiguousarray(np.asarray(W_v, dtype=np.float32))

    if _NC is None:
        _NC = build_bass()

    in_maps = []
    for core in range(8):
        b0 = core * B_PER_CORE
        in_maps.append(
            {
                "q": np.ascontiguousarray(q[b0 : b0 + B_PER_CORE]),
                "s": np.ascontiguousarray(s[b0 : b0 + B_PER_CORE]),
                "wqk": wqk,
                "wv": wv,
            }
        )
    res = run_bass_kernel_spmd(
        _NC, in_maps, core_ids=list(range(8)),
        trace=bool(int(os.environ.get("KTRACE", "0"))),
    )
    total = sum(float(r["out"][0, 0]) for r in res.results) / float(HW)
    kernel._last_results = res
    return np.asarray(total, dtype=np.float32)



# revision 2
# speedup vs baseline: 1.5942x; 1.5942x over previous
"""CrossTransformer kernel for Trainium2, data-parallel over batch across 8 cores.

Math per batch b (B=32, N=25, C=512, H=W=14, DK=DV=128):
  qq = Wqk @ Q    [128, 196]      qv = Wv @ Q     [128, 196]
  K  = Wqk @ S    [128, 4900]     V  = Wv @ S     [128, 4900]
  simT[nij, hw] = K^T @ qq        (computed directly in transposed layout)
  E = exp(simT)                   (no max subtraction; |sim| <~ 60 is safe in fp32)
  ctx_raw[hw, dv] = sum_nij E[nij, hw]^T @ V^T;  den[hw] = sum_nij E[nij, hw]
  ctx = ctx_raw / den
  partial += sum((qv^T - ctx)^2)
Output per core: scalar partial sum over its 4 batches; host sums and divides by H*W.

Pipeline: S streams in n-aligned [128, 980] fp32 DMA groups (784B descriptors,
the DMA-bandwidth floor). Attention work (V transpose, sim, exp, PV accumulate)
is emitted chunk-by-chunk as soon as projection tiles cover it, so every engine
runs concurrently with the DMA stream and the post-DMA tail is tiny.
"""

import os
import sys

sys.path.insert(0, "/opt/trn_rl_repo")

import numpy as np

import concourse.bass as bass
import concourse.bacc as bacc
import concourse.mybir as mybir
import concourse.tile as tile
from concourse.bass_utils import run_bass_kernel_spmd
from concourse.masks import make_identity

F32 = mybir.dt.float32
F32R = mybir.dt.float32r
BF16 = mybir.dt.bfloat16

B_PER_CORE = 4
N_SUP = 25
C = 512
HW = 196
NIJ = N_SUP * HW  # 4900
DK = 128
CCH = C // 128            # 4 c-chunks
GN = 5                    # support images per DMA group
GW = GN * HW              # 980 nij per group
NG = NIJ // GW            # 5 groups per batch
FT = 490                  # matmul tile width (2 per group, fits one PSUM bank)
NCH = (NIJ + 127) // 128  # 39 nij chunks of <=128
NPAIR = (NCH + 1) // 2    # 20 sim/exp pairs (19 full + 1 solo)


def build_bass():
    nc = bacc.Bacc(
        "TRN2", target_bir_lowering=False, debug=False, enable_asserts=False
    )
    q_d = nc.dram_tensor("q", [B_PER_CORE, C, HW], F32, kind="ExternalInput").ap()
    s_d = nc.dram_tensor(
        "s", [B_PER_CORE, N_SUP, C, HW], F32, kind="ExternalInput"
    ).ap()
    wqk_d = nc.dram_tensor("wqk", [DK, C], F32, kind="ExternalInput").ap()
    wv_d = nc.dram_tensor("wv", [DK, C], F32, kind="ExternalInput").ap()
    out_d = nc.dram_tensor("out", [1, 1], F32, kind="ExternalOutput").ap()

    with tile.TileContext(nc) as tc:
        with (
            tc.tile_pool(name="const", bufs=1) as const,
            tc.tile_pool(name="sg", bufs=12) as sg,
            tc.tile_pool(name="kvbf", bufs=4) as kvbf,
            tc.tile_pool(name="etp", bufs=8) as etp,
            tc.tile_pool(name="vtp", bufs=10) as vtp,
            tc.tile_pool(name="small", bufs=10) as small,
            tc.tile_pool(name="ps_proj", bufs=2, space="PSUM") as ps_proj,
            tc.tile_pool(name="ps_sim", bufs=2, space="PSUM") as ps_sim,
            tc.tile_pool(name="ps_vt", bufs=2, space="PSUM") as ps_vt,
            tc.tile_pool(name="ps_ctx", bufs=2, space="PSUM") as ps_ctx,
        ):
            # ---- constants / weights ----
            id_f32 = const.tile([128, 128], F32, tag="id_f32")
            make_identity(nc, id_f32)
            id_bf = const.tile([128, 128], BF16, tag="id_bf")
            make_identity(nc, id_bf)
            ones_bf = const.tile([128, 1], BF16, tag="ones_bf")
            nc.vector.memset(ones_bf, 1.0)

            wqk_sb = const.tile([128, C], F32, tag="wqk_sb")
            nc.sync.dma_start(out=wqk_sb, in_=wqk_d)
            wv_sb = const.tile([128, C], F32, tag="wv_sb")
            nc.sync.dma_start(out=wv_sb, in_=wv_d)

            # query load (before S groups: small, needed early for qq/qv)
            qsb = []
            for cc in range(CCH):
                qt = const.tile([128, B_PER_CORE * HW], F32R, tag=f"qsb{cc}")
                src = q_d[:, cc * 128 : (cc + 1) * 128, :].rearrange(
                    "b c ij -> c b ij"
                ).bitcast(F32R)
                nc.sync.dma_start(
                    out=qt.rearrange("p (b ij) -> p b ij", b=B_PER_CORE), in_=src
                )
                qsb.append(qt)

            wqkT = []
            wvT = []
            for cc in range(CCH):
                for (src, dstl, nm) in ((wqk_sb, wqkT, "qk"), (wv_sb, wvT, "v")):
                    pt = ps_vt.tile([128, 128], F32, tag="ps_vt")
                    nc.tensor.transpose(pt, src[:, cc * 128 : (cc + 1) * 128], id_f32)
                    wt = const.tile([128, 128], F32R, tag=f"w{nm}T{cc}")
                    nc.vector.tensor_copy(wt, pt)
                    dstl.append(wt)

            # ---- query projections (all 4 batches at once) ----
            qq_bf = const.tile([128, B_PER_CORE * HW], BF16, tag="qq_bf")
            qv_sb = const.tile([128, B_PER_CORE * HW], F32, tag="qv_sb")
            for wT, dst in ((wqkT, qq_bf), (wvT, qv_sb)):
                for half in range(2):
                    hw0 = half * 392
                    pq = ps_proj.tile([128, FT], F32, tag="ps_proj")
                    for cc in range(CCH):
                        nc.tensor.matmul(
                            pq[:, :392],
                            lhsT=wT[cc],
                            rhs=qsb[cc][:, hw0 : hw0 + 392],
                            start=(cc == 0),
                            stop=(cc == CCH - 1),
                        )
                    nc.vector.tensor_copy(dst[:, hw0 : hw0 + 392], pq[:, :392])

            # qv^T per (b, hw-chunk): [hw<=128, 128] fp32 — matches ctx layout
            qvT = {}
            for b in range(B_PER_CORE):
                for h in range(2):
                    hww = 128 if h == 0 else HW - 128
                    pt = ps_vt.tile([128, 128], F32, tag="ps_vt")
                    nc.tensor.transpose(
                        pt[:hww, :],
                        qv_sb[:, b * HW + h * 128 : b * HW + h * 128 + hww],
                        id_f32,
                    )
                    qt = const.tile([128, 128], F32, tag=f"qvT{b}_{h}")
                    nc.vector.tensor_copy(qt[:hww, :], pt[:hww, :])
                    qvT[(b, h)] = qt

            partials = const.tile([128, 2 * B_PER_CORE], F32, tag="partials")
            nc.vector.memset(partials, 0.0)

            # ---- per-batch interleaved pipeline ----
            for b in range(B_PER_CORE):
                k_bf = kvbf.tile([128, NIJ], BF16, tag="k_bf")
                v_bf = kvbf.tile([128, NIJ], BF16, tag="v_bf")
                # both h ctx+den accumulators packed in one PSUM bank:
                # cols [h*129, h*129+128) = ctx, col h*129+128 = denom
                pc = ps_ctx.tile([128, 258], F32, tag="ps_ctx")

                state = {"nB": 0, "nC": 0, "nD": 0}
                vt_tiles = [None] * NCH
                et_tiles = [None] * NPAIR

                def emit_attn(cov, b=b, k_bf=k_bf, v_bf=v_bf, pc=pc,
                              state=state, vt_tiles=vt_tiles, et_tiles=et_tiles):
                    # B: V^T chunks via PE transpose
                    while state["nB"] < NCH and min(128 * (state["nB"] + 1), NIJ) <= cov:
                        j = state["nB"]
                        cw = min(128, NIJ - j * 128)
                        pt = ps_vt.tile([128, 128], BF16, tag="ps_vt")
                        nc.tensor.transpose(
                            pt[:cw, :], v_bf[:, j * 128 : j * 128 + cw], id_bf
                        )
                        vt = vtp.tile([128, 128], BF16, tag="vt")
                        nc.vector.tensor_copy(vt[:cw, :], pt[:cw, :])
                        vt_tiles[j] = vt
                        state["nB"] += 1
                    # C: sim pairs -> one exp per pair
                    while state["nC"] < NPAIR and min(256 * (state["nC"] + 1), NIJ) <= cov:
                        p = state["nC"]
                        ps = ps_sim.tile([128, 392], F32, tag="ps_sim")
                        solo = 2 * p + 1 >= NCH
                        for s in range(1 if solo else 2):
                            j = 2 * p + s
                            cw = min(128, NIJ - j * 128)
                            nc.tensor.matmul(
                                ps[:cw, s * HW : (s + 1) * HW],
                                lhsT=k_bf[:, j * 128 : j * 128 + cw],
                                rhs=qq_bf[:, b * HW : (b + 1) * HW],
                                start=True,
                                stop=True,
                            )
                        e = etp.tile([128, 392], BF16, tag="et")
                        if solo:
                            cw = NIJ - (2 * p) * 128
                            nc.vector.memset(e, 0.0)
                            nc.scalar.activation(
                                out=e[:cw, 0:HW],
                                in_=ps[:cw, 0:HW],
                                func=mybir.ActivationFunctionType.Exp,
                            )
                        else:
                            nc.scalar.activation(
                                out=e,
                                in_=ps,
                                func=mybir.ActivationFunctionType.Exp,
                            )
                        et_tiles[p] = e
                        state["nC"] += 1
                    # D: PV accumulation (ctx + denom streams per h)
                    avail = min(min(2 * state["nC"], NCH), state["nB"])
                    while state["nD"] < avail:
                        j = state["nD"]
                        e = et_tiles[j // 2]
                        c0 = (j % 2) * HW
                        for h in range(2):
                            hww = 128 if h == 0 else HW - 128
                            lhs = e[:, c0 + h * 128 : c0 + h * 128 + hww]
                            nc.tensor.matmul(
                                pc[:hww, h * 129 : h * 129 + 128],
                                lhsT=lhs,
                                rhs=vt_tiles[j],
                                start=(j == 0),
                                stop=(j == NCH - 1),
                            )
                            nc.tensor.matmul(
                                pc[:hww, h * 129 + 128 : h * 129 + 129],
                                lhsT=lhs,
                                rhs=ones_bf,
                                start=(j == 0),
                                stop=(j == NCH - 1),
                            )
                        state["nD"] += 1

                for g in range(NG):
                    sgt = []
                    for cc in range(CCH):
                        s_t = sg.tile([128, GW], F32R, tag="s_t")
                        src = s_d[
                            b, g * GN : (g + 1) * GN,
                            cc * 128 : (cc + 1) * 128, :,
                        ].rearrange("n c ij -> c n ij").bitcast(F32R)
                        nc.sync.dma_start(
                            out=s_t.rearrange("p (n ij) -> p n ij", n=GN),
                            in_=src,
                        )
                        sgt.append(s_t)
                    for half in range(2):
                        c0 = g * GW + half * FT
                        pk = ps_proj.tile([128, FT], F32, tag="ps_proj")
                        for cc in range(CCH):
                            nc.tensor.matmul(
                                pk,
                                lhsT=wqkT[cc],
                                rhs=sgt[cc][:, half * FT : (half + 1) * FT],
                                start=(cc == 0),
                                stop=(cc == CCH - 1),
                            )
                        nc.vector.tensor_copy(k_bf[:, c0 : c0 + FT], pk)
                        pv = ps_proj.tile([128, FT], F32, tag="ps_proj")
                        for cc in range(CCH):
                            nc.tensor.matmul(
                                pv,
                                lhsT=wvT[cc],
                                rhs=sgt[cc][:, half * FT : (half + 1) * FT],
                                start=(cc == 0),
                                stop=(cc == CCH - 1),
                            )
                        nc.scalar.copy(v_bf[:, c0 : c0 + FT], pv)
                        emit_attn(c0 + FT)

                # drain ctx accumulators -> partial sums
                for h in range(2):
                    hww = 128 if h == 0 else HW - 128
                    r = small.tile([128, 1], F32, tag="recip")
                    nc.vector.reciprocal(
                        r[:hww], pc[:hww, h * 129 + 128 : h * 129 + 129]
                    )
                    ctx = small.tile([128, 128], F32, tag="ctx")
                    nc.vector.tensor_scalar_mul(
                        ctx[:hww, :], pc[:hww, h * 129 : h * 129 + 128], r[:hww]
                    )
                    d = small.tile([128, 128], F32, tag="diff")
                    nc.vector.tensor_sub(
                        d[:hww, :], qvT[(b, h)][:hww, :], ctx[:hww, :]
                    )
                    d2 = small.tile([128, 128], F32, tag="d2")
                    nc.vector.tensor_mul(d2[:hww, :], d[:hww, :], d[:hww, :])
                    nc.vector.reduce_sum(
                        partials[:hww, 2 * b + h : 2 * b + h + 1],
                        d2[:hww, :],
                        axis=mybir.AxisListType.X,
                    )

            # ---- final reduction to scalar ----
            tot = small.tile([128, 1], F32, tag="tot")
            nc.vector.reduce_sum(tot, partials, axis=mybir.AxisListType.X)
            ones = small.tile([128, 1], F32, tag="ones")
            nc.vector.memset(ones, 1.0)
            pf = ps_vt.tile([128, 128], F32, tag="ps_vt")
            nc.tensor.matmul(pf[0:1, 0:1], lhsT=tot, rhs=ones, start=True, stop=True)
            ob = small.tile([1, 1], F32, tag="ob")
            nc.vector.tensor_copy(ob, pf[0:1, 0:1])
            nc.sync.dma_start(out=out_d, in_=ob)

    nc.compile()
    return nc


_NC = None


def kernel(query_repr, supports_repr, W_qk, W_v):
    global _NC
    q = np.ascontiguousarray(np.asarray(query_repr, dtype=np.float32)).reshape(
        32, C, HW
    )
    s = np.ascontiguousarray(np.asarray(supports_repr, dtype=np.float32)).reshape(
        32, N_SUP, C, HW
    )
    wqk = np.ascontiguousarray(np.asarray(W_qk, dtype=np.float32))
    wv = np.ascontiguousarray(np.asarray(W_v, dtype=np.float32))

    if _NC is None:
        _NC = build_bass()

    in_maps = []
    for core in range(8):
        b0 = core * B_PER_CORE
        in_maps.append(
            {
                "q": np.ascontiguousarray(q[b0 : b0 + B_PER_CORE]),
                "s": np.ascontiguousarray(s[b0 : b0 + B_PER_CORE]),
                "wqk": wqk,
                "wv": wv,
            }
        )
    res = run_bass_kernel_spmd(
        _NC, in_maps, core_ids=list(range(8)),
        trace=bool(int(os.environ.get("KTRACE", "0"))),
    )
    total = sum(float(r["out"][0, 0]) for r in res.results) / float(HW)
    kernel._last_results = res
    return np.asarray(total, dtype=np.float32)


# revision 9
# speedup vs baseline: 1.6222x; 1.0176x over previous
"""CrossTransformer kernel for Trainium2, data-parallel over batch across 8 cores.

Math per batch b (B=32, N=25, C=512, H=W=14, DK=DV=128):
  qq = Wqk @ Q    [128, 196]      qv = Wv @ Q     [128, 196]
  K  = Wqk @ S    [128, 4900]     V  = Wv @ S     [128, 4900]
  simT[nij, hw] = K^T @ qq        (computed directly in transposed layout)
  E = exp(simT)                   (no max subtraction; |sim| <~ 60 is safe in fp32)
  ctx_raw[hw, dv] = sum_nij E[nij, hw]^T @ V^T;  den[hw] = sum_nij E[nij, hw]
  ctx = ctx_raw / den
  partial += sum((qv^T - ctx)^2)
Output per core: scalar partial sum over its 4 batches; host sums and divides by H*W.

Pipeline: S streams in n-aligned [128, 980] fp32 DMA groups (784B descriptors,
the DMA-bandwidth floor). Attention work (V transpose, sim, exp, PV accumulate)
is emitted chunk-by-chunk as soon as projection tiles cover it, so every engine
runs concurrently with the DMA stream and the post-DMA tail is tiny.
"""

import os
import sys

sys.path.insert(0, "/opt/trn_rl_repo")

import numpy as np

import concourse.bass as bass
import concourse.bacc as bacc
import concourse.mybir as mybir
import concourse.tile as tile
from concourse.bass_utils import run_bass_kernel_spmd
from concourse.masks import make_identity

F32 = mybir.dt.float32
F32R = mybir.dt.float32r
BF16 = mybir.dt.bfloat16

B_PER_CORE = 4
N_SUP = 25
C = 512
HW = 196
NIJ = N_SUP * HW  # 4900
DK = 128
CCH = C // 128            # 4 c-chunks
GN = 5                    # support images per DMA group
GW = GN * HW              # 980 nij per group
NG = NIJ // GW            # 5 groups per batch
FT = 490                  # matmul tile width (2 per group, fits one PSUM bank)
NCH = (NIJ + 127) // 128  # 39 nij chunks of <=128
NPAIR = (NCH + 1) // 2    # 20 sim/exp pairs (19 full + 1 solo)


def build_bass():
    nc = bacc.Bacc(
        "TRN2", target_bir_lowering=False, debug=False, enable_asserts=False
    )
    q_d = nc.dram_tensor("q", [B_PER_CORE, C, HW], F32, kind="ExternalInput").ap()
    s_d = nc.dram_tensor(
        "s", [B_PER_CORE, N_SUP, C, HW], F32, kind="ExternalInput"
    ).ap()
    wqk_d = nc.dram_tensor("wqk", [DK, C], F32, kind="ExternalInput").ap()
    wv_d = nc.dram_tensor("wv", [DK, C], F32, kind="ExternalInput").ap()
    out_d = nc.dram_tensor("out", [1, 1], F32, kind="ExternalOutput").ap()

    with tile.TileContext(nc) as tc:
        with (
            tc.tile_pool(name="const", bufs=1) as const,
            tc.tile_pool(name="sg", bufs=16) as sg,
            tc.tile_pool(name="kvbf", bufs=4) as kvbf,
            tc.tile_pool(name="etp", bufs=8) as etp,
            tc.tile_pool(name="vtp", bufs=10) as vtp,
            tc.tile_pool(name="small", bufs=10) as small,
            tc.tile_pool(name="ps_proj", bufs=2, space="PSUM") as ps_proj,
            tc.tile_pool(name="ps_sim", bufs=2, space="PSUM") as ps_sim,
            tc.tile_pool(name="ps_vt", bufs=2, space="PSUM") as ps_vt,
            tc.tile_pool(name="ps_ctx", bufs=2, space="PSUM") as ps_ctx,
        ):
            # ---- constants / weights ----
            id_f32 = const.tile([128, 128], F32, tag="id_f32")
            make_identity(nc, id_f32)
            id_bf = const.tile([128, 128], BF16, tag="id_bf")
            make_identity(nc, id_bf)
            ones_bf = const.tile([128, 1], BF16, tag="ones_bf")
            nc.vector.memset(ones_bf, 1.0)

            wqk_sb = const.tile([128, C], F32, tag="wqk_sb")
            nc.sync.dma_start(out=wqk_sb, in_=wqk_d)
            wv_sb = const.tile([128, C], F32, tag="wv_sb")
            nc.sync.dma_start(out=wv_sb, in_=wv_d)

            # query load (before S groups: small, needed early for qq/qv)
            qsb = []
            for cc in range(CCH):
                qt = const.tile([128, B_PER_CORE * HW], F32R, tag=f"qsb{cc}")
                src = q_d[:, cc * 128 : (cc + 1) * 128, :].rearrange(
                    "b c ij -> c b ij"
                ).bitcast(F32R)
                nc.sync.dma_start(
                    out=qt.rearrange("p (b ij) -> p b ij", b=B_PER_CORE), in_=src
                )
                qsb.append(qt)

            wqkT = []
            wvT = []
            for cc in range(CCH):
                for (src, dstl, nm) in ((wqk_sb, wqkT, "qk"), (wv_sb, wvT, "v")):
                    pt = ps_vt.tile([128, 128], F32, tag="ps_vt")
                    nc.tensor.transpose(pt, src[:, cc * 128 : (cc + 1) * 128], id_f32)
                    wt = const.tile([128, 128], F32R, tag=f"w{nm}T{cc}")
                    nc.vector.tensor_copy(wt, pt)
                    dstl.append(wt)

            # ---- query projections (all 4 batches at once) ----
            qq_bf = const.tile([128, B_PER_CORE * HW], BF16, tag="qq_bf")
            qv_sb = const.tile([128, B_PER_CORE * HW], F32, tag="qv_sb")
            for wT, dst in ((wqkT, qq_bf), (wvT, qv_sb)):
                for half in range(2):
                    hw0 = half * 392
                    pq = ps_proj.tile([128, FT], F32, tag="ps_proj")
                    for cc in range(CCH):
                        nc.tensor.matmul(
                            pq[:, :392],
                            lhsT=wT[cc],
                            rhs=qsb[cc][:, hw0 : hw0 + 392],
                            start=(cc == 0),
                            stop=(cc == CCH - 1),
                        )
                    nc.vector.tensor_copy(dst[:, hw0 : hw0 + 392], pq[:, :392])

            # qv^T per (b, hw-chunk): [hw<=128, 128] fp32 — matches ctx layout
            qvT = {}
            for b in range(B_PER_CORE):
                for h in range(2):
                    hww = 128 if h == 0 else HW - 128
                    pt = ps_vt.tile([128, 128], F32, tag="ps_vt")
                    nc.tensor.transpose(
                        pt[:hww, :],
                        qv_sb[:, b * HW + h * 128 : b * HW + h * 128 + hww],
                        id_f32,
                    )
                    qt = const.tile([128, 128], F32, tag=f"qvT{b}_{h}")
                    nc.vector.tensor_copy(qt[:hww, :], pt[:hww, :])
                    qvT[(b, h)] = qt

            partials = const.tile([128, 2 * B_PER_CORE], F32, tag="partials")
            nc.vector.memset(partials, 0.0)

            # ---- per-batch interleaved pipeline ----
            for b in range(B_PER_CORE):
                k_bf = kvbf.tile([128, NIJ], BF16, tag="k_bf")
                v_bf = kvbf.tile([128, NIJ], BF16, tag="v_bf")
                # both h ctx+den accumulators in one bank-aligned PSUM tile:
                # cols [h*256, h*256+128) = ctx, col h*256+128 = denom
                pc = ps_ctx.tile([128, 512], F32, tag="ps_ctx")

                state = {"nB": 0, "nC": 0, "nD": 0}
                vt_tiles = [None] * NCH
                et_tiles = [None] * NPAIR

                def emit_attn(cov, b=b, k_bf=k_bf, v_bf=v_bf, pc=pc,
                              state=state, vt_tiles=vt_tiles, et_tiles=et_tiles):
                    # B: V^T chunks via PE transpose
                    while state["nB"] < NCH and min(128 * (state["nB"] + 1), NIJ) <= cov:
                        j = state["nB"]
                        cw = min(128, NIJ - j * 128)
                        pt = ps_vt.tile([128, 128], BF16, tag="ps_vt")
                        nc.tensor.transpose(
                            pt[:cw, :], v_bf[:, j * 128 : j * 128 + cw], id_bf
                        )
                        vt = vtp.tile([128, 128], BF16, tag="vt")
                        nc.vector.tensor_copy(vt[:cw, :], pt[:cw, :])
                        vt_tiles[j] = vt
                        state["nB"] += 1
                    # C: sim pairs -> one exp per pair
                    while state["nC"] < NPAIR and min(256 * (state["nC"] + 1), NIJ) <= cov:
                        p = state["nC"]
                        ps = ps_sim.tile([128, 392], F32, tag="ps_sim")
                        solo = 2 * p + 1 >= NCH
                        for s in range(1 if solo else 2):
                            j = 2 * p + s
                            cw = min(128, NIJ - j * 128)
                            nc.tensor.matmul(
                                ps[:cw, s * HW : (s + 1) * HW],
                                lhsT=k_bf[:, j * 128 : j * 128 + cw],
                                rhs=qq_bf[:, b * HW : (b + 1) * HW],
                                start=True,
                                stop=True,
                            )
                        e = etp.tile([128, 392], BF16, tag="et")
                        if solo:
                            cw = NIJ - (2 * p) * 128
                            nc.vector.memset(e, 0.0)
                            nc.scalar.activation(
                                out=e[:cw, 0:HW],
                                in_=ps[:cw, 0:HW],
                                func=mybir.ActivationFunctionType.Exp,
                            )
                        else:
                            nc.scalar.activation(
                                out=e,
                                in_=ps,
                                func=mybir.ActivationFunctionType.Exp,
                            )
                        et_tiles[p] = e
                        state["nC"] += 1
                    # D: PV accumulation (ctx + denom streams per h)
                    avail = min(min(2 * state["nC"], NCH), state["nB"])
                    while state["nD"] < avail:
                        j = state["nD"]
                        e = et_tiles[j // 2]
                        c0 = (j % 2) * HW
                        for h in range(2):
                            hww = 128 if h == 0 else HW - 128
                            lhs = e[:, c0 + h * 128 : c0 + h * 128 + hww]
                            nc.tensor.matmul(
                                pc[:hww, h * 256 : h * 256 + 128],
                                lhsT=lhs,
                                rhs=vt_tiles[j],
                                start=(j == 0),
                                stop=(j == NCH - 1),
                            )
                            nc.tensor.matmul(
                                pc[:hww, h * 256 + 128 : h * 256 + 129],
                                lhsT=lhs,
                                rhs=ones_bf,
                                start=(j == 0),
                                stop=(j == NCH - 1),
                            )
                        state["nD"] += 1

                for g in range(NG):
                    sgt = []
                    for cc in range(CCH):
                        s_t = sg.tile([128, GW], F32R, tag="s_t")
                        src = s_d[
                            b, g * GN : (g + 1) * GN,
                            cc * 128 : (cc + 1) * 128, :,
                        ].rearrange("n c ij -> c n ij").bitcast(F32R)
                        nc.sync.dma_start(
                            out=s_t.rearrange("p (n ij) -> p n ij", n=GN),
                            in_=src,
                        )
                        sgt.append(s_t)
                    for half in range(2):
                        c0 = g * GW + half * FT
                        pk = ps_proj.tile([128, FT], F32, tag="ps_proj")
                        for cc in range(CCH):
                            nc.tensor.matmul(
                                pk,
                                lhsT=wqkT[cc],
                                rhs=sgt[cc][:, half * FT : (half + 1) * FT],
                                start=(cc == 0),
                                stop=(cc == CCH - 1),
                            )
                        nc.vector.tensor_copy(k_bf[:, c0 : c0 + FT], pk)
                        pv = ps_proj.tile([128, FT], F32, tag="ps_proj")
                        for cc in range(CCH):
                            nc.tensor.matmul(
                                pv,
                                lhsT=wvT[cc],
                                rhs=sgt[cc][:, half * FT : (half + 1) * FT],
                                start=(cc == 0),
                                stop=(cc == CCH - 1),
                            )
                        nc.scalar.copy(v_bf[:, c0 : c0 + FT], pv)
                        emit_attn(c0)  # one half-tile behind: copies settled
                emit_attn(NIJ)  # flush the lagged final half

                # drain ctx accumulators -> partial sums
                # d = ctx - qv = (pc * recip) - qvT; partial = sum(d*d)
                for h in range(2):
                    hww = 128 if h == 0 else HW - 128
                    r = small.tile([128, 1], F32, tag="recip")
                    nc.vector.reciprocal(
                        r[:hww], pc[:hww, h * 256 + 128 : h * 256 + 129]
                    )
                    d = small.tile([128, 128], F32, tag="diff")
                    nc.vector.scalar_tensor_tensor(
                        out=d[:hww, :],
                        in0=pc[:hww, h * 256 : h * 256 + 128],
                        scalar=r[:hww],
                        in1=qvT[(b, h)][:hww, :],
                        op0=mybir.AluOpType.mult,
                        op1=mybir.AluOpType.subtract,
                    )
                    d2 = small.tile([128, 128], F32, tag="d2")
                    nc.vector.scalar_tensor_tensor(
                        out=d2[:hww, :],
                        in0=d[:hww, :],
                        scalar=1.0,
                        in1=d[:hww, :],
                        op0=mybir.AluOpType.mult,
                        op1=mybir.AluOpType.mult,
                        accum_out=partials[:hww, 2 * b + h : 2 * b + h + 1],
                    )

            # ---- final reduction to scalar ----
            tot = small.tile([128, 1], F32, tag="tot")
            nc.vector.reduce_sum(tot, partials, axis=mybir.AxisListType.X)
            ones = small.tile([128, 1], F32, tag="ones")
            nc.vector.memset(ones, 1.0)
            pf = ps_vt.tile([128, 128], F32, tag="ps_vt")
            nc.tensor.matmul(pf[0:1, 0:1], lhsT=tot, rhs=ones, start=True, stop=True)
            ob = small.tile([1, 1], F32, tag="ob")
            nc.vector.tensor_copy(ob, pf[0:1, 0:1])
            nc.sync.dma_start(out=out_d, in_=ob)

    nc.compile()
    return nc


_NC = None


def kernel(query_repr, supports_repr, W_qk, W_v):
    global _NC
    q = np.ascontiguousarray(np.asarray(query_repr, dtype=np.float32)).reshape(
        32, C, HW
    )
    s = np.ascontiguousarray(np.asarray(supports_repr, dtype=np.float32)).reshape(
        32, N_SUP, C, HW
    )
    wqk = np.ascontiguousarray(np.asarray(W_qk, dtype=np.float32))
    wv = np.ascontiguousarray(np.asarray(W_v, dtype=np.float32))

    if _NC is None:
        _NC = build_bass()

    in_maps = []
    for core in range(8):
        b0 = core * B_PER_CORE
        in_maps.append(
            {
                "q": np.ascontiguousarray(q[b0 : b0 + B_PER_CORE]),
                "s": np.ascontiguousarray(s[b0 : b0 + B_PER_CORE]),
                "wqk": wqk,
                "wv": wv,
            }
        )
    res = run_bass_kernel_spmd(
        _NC, in_maps, core_ids=list(range(8)),
        trace=bool(int(os.environ.get("KTRACE", "0"))),
    )
    total = sum(float(r["out"][0, 0]) for r in res.results) / float(HW)
    kernel._last_results = res
    return np.asarray(total, dtype=np.float32)
